# revision 7
# baseline (speedup 1.0000x reference)
"""Batched graph-conv (diffusion GCN step) kernel for 8 TRN2 NeuronCores.

Math per batch b (B=64, N=512, D=64, S=2, MAX_STEP=2, M=5 metrics):
    m0 = x0
    m1 = A0 @ x0
    m2 = 2*A0 @ m1 - m0
    m3 = A1 @ m2
    m4 = 2*A1 @ m3 - m1
    out = sum_k  m_k @ W_k + b     where W_k = W_out.reshape(D,5,OUT)[:,k,:]

Sharding: batch dim across the 8 cores (8 batches/core); weights replicated.

Device dataflow (per batch):
  - A fed pre-transposed+tiled from host: aT[s,b][p, kc, i] = A[i, kc*128+p]
    so each k-chunk kc is a ready-to-stream matmul rhs [128(j), 512(i)].
  - state kept in BOTH forms: natural x[j,d] (matmul stationary operand)
    and transposed xT[d,i] (product-result layout + projection stream).
  - product:  psum[d,i] = sum_kc  x_nat[:,kc*D:].T @ aT[:,kc,:]   (f32r)
  - PE-transpose (identity trick) regenerates the natural form of m1..m3.
  - per-batch projection runs as one tight 5-matmul PSUM accumulation
    at the end (bank alive ~1us instead of the whole batch).
  - 4 batches in flight; per pipeline stage all 4 batches' product
    matmuls are emitted back-to-back to keep the PE dense (HAM warm);
    a dummy-transpose warmup covers the initial DMA window.
"""

import sys

import numpy as np

sys.path.insert(0, "/opt/trn_rl_repo")

from contextlib import ExitStack

import concourse.bass as bass
import concourse.mybir as mybir
import concourse.tile as tile
from concourse import bacc
from concourse import bass_utils

P = 128          # SBUF partitions / matmul contraction tile
N = 512          # graph nodes
D = 64           # feature dim
OUT = 64         # output dim
S = 2            # supports
NMET = 5         # metrics
B = 64           # global batch
NCORES = 8
BPC = B // NCORES  # batches per core
KC = N // P        # k-chunks per product
GRP = 8            # batches in flight (single stage-major group)

F32 = mybir.dt.float32
# f32r: fp32 storage, single-pass PE matmul (4x faster than fp32's 2-pass
# half-rate path when the moving dim is >= 256). Walrus requires every
# tensor consumed by an FP32r matmul to be *produced* as FP32r, so all
# matmul-feeding tensors (DRAM + SBUF) are declared float32r end-to-end.
RDT = mybir.dt.float32r


def build_nc():
    nc = bacc.Bacc("TRN2", target_bir_lowering=False, debug=False)

    xT_d = nc.dram_tensor("xT", [BPC, D, N], RDT, kind="ExternalInput").ap()
    xn_d = nc.dram_tensor("xnat", [BPC, P, KC * D], RDT, kind="ExternalInput").ap()
    a_d = nc.dram_tensor("aT", [S, BPC, P, KC, N], RDT, kind="ExternalInput").ap()
    w_d = nc.dram_tensor("w", [D, NMET * OUT], RDT, kind="ExternalInput").ap()
    b_d = nc.dram_tensor("bias", [OUT, 1], F32, kind="ExternalInput").ap()
    i_d = nc.dram_tensor("ident", [P, P], RDT, kind="ExternalInput").ap()
    o_d = nc.dram_tensor("outT", [BPC, OUT, N], F32, kind="ExternalOutput").ap()

    with ExitStack() as ctx:
        tc = ctx.enter_context(tile.TileContext(nc))
        _emit(nc, tc, ctx, xT_d, xn_d, a_d, w_d, b_d, i_d, o_d)

    nc.compile()
    return nc


def _emit(nc, tc, ctx, xT_d, xn_d, a_d, w_d, b_d, i_d, o_d):
    a_pool = ctx.enter_context(tc.tile_pool(name="a", bufs=12))
    stT = ctx.enter_context(tc.tile_pool(name="stT", bufs=5))
    nat = ctx.enter_context(tc.tile_pool(name="nat", bufs=5))
    outsb = ctx.enter_context(tc.tile_pool(name="outsb", bufs=3))
    const = ctx.enter_context(tc.tile_pool(name="const", bufs=1))
    ps_mm = ctx.enter_context(tc.tile_pool(name="psmm", bufs=4, space="PSUM"))
    ps_tr = ctx.enter_context(tc.tile_pool(name="pstr", bufs=2, space="PSUM"))
    ps_out = ctx.enter_context(tc.tile_pool(name="psout", bufs=2, space="PSUM"))

    w_sb = const.tile([D, NMET * OUT], RDT)
    nc.sync.dma_start(w_sb[:], w_d)
    bias_sb = const.tile([OUT, 1], F32)
    nc.sync.dma_start(bias_sb[:], b_d)
    ident = const.tile([P, P], RDT)
    nc.sync.dma_start(ident[:], i_d)

    def product(a_tile, src_nat, tag):
        """psum[d, i] = sum_j A[i, j] * x[j, d]  (result transposed)."""
        ps = ps_mm.tile([D, N], F32, tag="ps", name=f"ps_{tag}")
        for kc in range(KC):
            nc.tensor.matmul(
                ps[:],
                src_nat[:, kc * D:(kc + 1) * D],
                a_tile[:, kc, :],
                start=(kc == 0),
                stop=(kc == KC - 1),
            )
        return ps

    def transposes(mT, tag):
        """[D, N] transposed state -> PSUM [P, KC*D] natural (packed)."""
        pt = ps_tr.tile([P, KC * D], RDT, tag="pt", name=f"pt_{tag}")
        for kc in range(KC):
            nc.tensor.matmul(
                pt[:, kc * D:(kc + 1) * D],
                mT[:, kc * P:(kc + 1) * P],
                ident[:D, :D],
                is_transpose=True,
                skip_group_check=True,
            )
        return pt

    def evict_nat(pt, name):
        m_nat = nat.tile([P, KC * D], RDT, tag="nat", name=name, bufs=16)
        nc.scalar.copy(m_nat[:], pt[:])
        return m_nat

    # ---- PE warmup: dummy transposes on the identity while the first A
    # tiles stream in, so the HAM un-throttles before real work starts.
    # (borrows an out-psum slot, released before the first projection)
    warm_ps = ps_out.tile([P, KC * D], RDT, tag="outp", name="warm")
    for _ in range(48):
        nc.tensor.matmul(
            warm_ps[:, 0:D], ident[:D, 0:P], ident[:D, :D],
            is_transpose=True, skip_group_check=True,
        )

    # ---- all input DMAs, emitted upfront in exact use order; the tile
    # pools' buf limits turn this into a continuous, self-paced prefetch
    # stream with no compute-emission ordering constraints.
    loads = {}
    for b in range(BPC):
        t = stT.tile([D, N], RDT, tag="x0T", name=f"x0T_{b}", bufs=BPC)
        nc.sync.dma_start(t[:], xT_d[b])
        loads[("x", b)] = t
        t = nat.tile([P, KC * D], RDT, tag="x0n", name=f"x0n_{b}", bufs=BPC)
        nc.sync.dma_start(t[:], xn_d[b])
        loads[("xn", b)] = t
    for g in range(0, BPC, GRP):
        for s in range(S):
            for b in range(g, g + GRP):
                t = a_pool.tile([P, KC, N], RDT, tag="A", name=f"a{s}_{b}")
                nc.sync.dma_start(t[:], a_d[s, b])
                loads[("a", s, b)] = t

    def batch_stages(b):
        st = {}

        def load():
            st["x0T"] = loads[("x", b)]
            st["x0n"] = loads[("xn", b)]
            st["a0"] = loads[("a", 0, b)]
            st["a1"] = loads[("a", 1, b)]

        def p1():
            ps = product(st["a0"], st["x0n"], f"1_{b}")
            st["m1T"] = stT.tile([D, N], RDT, tag="m1T", name=f"m1T_{b}", bufs=BPC)
            nc.vector.tensor_copy(st["m1T"][:], ps[:])

        def t1():
            st["pt1"] = transposes(st["m1T"], f"1_{b}")
            st["m1n"] = evict_nat(st["pt1"], f"m1n_{b}")

        def p2():
            ps = product(st["a0"], st["m1n"], f"2_{b}")
            st["m2T"] = stT.tile([D, N], RDT, tag="m2T", name=f"m2T_{b}", bufs=BPC)
            nc.vector.scalar_tensor_tensor(
                st["m2T"][:], ps[:], 2.0, st["x0T"][:],
                op0=mybir.AluOpType.mult, op1=mybir.AluOpType.subtract,
            )

        def t2():
            st["pt2"] = transposes(st["m2T"], f"2_{b}")
            st["m2n"] = evict_nat(st["pt2"], f"m2n_{b}")

        def p3():
            ps = product(st["a1"], st["m2n"], f"3_{b}")
            st["m3T"] = stT.tile([D, N], RDT, tag="m3T", name=f"m3T_{b}", bufs=BPC)
            nc.vector.tensor_copy(st["m3T"][:], ps[:])

        def t3():
            st["pt3"] = transposes(st["m3T"], f"3_{b}")
            st["m3n"] = evict_nat(st["pt3"], f"m3n_{b}")

        def p4():
            ps = product(st["a1"], st["m3n"], f"4_{b}")
            st["m4T"] = stT.tile([D, N], RDT, tag="m4T", name=f"m4T_{b}", bufs=4)
            nc.vector.scalar_tensor_tensor(
                st["m4T"][:], ps[:], 2.0, st["m1T"][:],
                op0=mybir.AluOpType.mult, op1=mybir.AluOpType.subtract,
            )
            prj()

        def prj():
            outp = ps_out.tile([OUT, N], F32, tag="outp", name=f"outp_{b}")
            mts = [st["x0T"], st["m1T"], st["m2T"], st["m3T"], st["m4T"]]
            for k in range(NMET):
                nc.tensor.matmul(
                    outp[:],
                    w_sb[:, k * OUT:(k + 1) * OUT],
                    mts[k][:],
                    start=(k == 0),
                    stop=(k == NMET - 1),
                )
            o_sb = outsb.tile([OUT, N], F32, tag="o", name=f"o_{b}")
            nc.scalar.add(o_sb[:], outp[:], bias_sb[:])
            nc.gpsimd.dma_start(o_d[b], o_sb[:])

        return [load, p1, t1, p2, t2, p3, t3, p4]

    NSTAGE = 8
    for g in range(0, BPC, GRP):
        group = [batch_stages(b) for b in range(g, g + GRP)]
        for si in range(NSTAGE):
            for stages in group:
                stages[si]()


_NC_CACHE = None


def _get_nc():
    global _NC_CACHE
    if _NC_CACHE is None:
        _NC_CACHE = build_nc()
    return _NC_CACHE


def shard_inputs(inputs, supports, W_out, b_out):
    """Host-side shard + layout prep. Returns list of per-core input dicts."""
    inputs = np.ascontiguousarray(inputs, dtype=np.float32)
    supports = np.ascontiguousarray(supports, dtype=np.float32)
    w = np.ascontiguousarray(W_out.reshape(D, NMET * OUT), dtype=np.float32)
    bias = np.ascontiguousarray(b_out.reshape(OUT, 1), dtype=np.float32)
    ident = np.eye(P, dtype=np.float32)

    in_maps = []
    for c in range(NCORES):
        sl = slice(c * BPC, (c + 1) * BPC)
        x = inputs[sl]                                   # [BPC, N, D]
        xT = np.ascontiguousarray(x.transpose(0, 2, 1))  # [BPC, D, N]
        # natural packed: [p, kc*D + d] = x[kc*128 + p, d]
        xn = np.ascontiguousarray(
            x.reshape(BPC, KC, P, D).transpose(0, 2, 1, 3).reshape(BPC, P, KC * D)
        )
        a = supports[:, sl]                              # [S, BPC, N(i), N(j)]
        # aT[s,b][p, kc, i] = A[i, kc*128+p]
        aT = np.ascontiguousarray(
            a.transpose(0, 1, 3, 2)                      # [S, BPC, j, i]
            .reshape(S, BPC, KC, P, N)
            .transpose(0, 1, 3, 2, 4)                    # [S, BPC, p, kc, i]
        )
        in_maps.append({"xT": xT, "xnat": xn, "aT": aT, "w": w, "bias": bias,
                        "ident": ident})
    return in_maps


def run(inputs, supports, W_out, b_out, trace=False, trace_cores=None):
    nc = _get_nc()
    in_maps = shard_inputs(inputs, supports, W_out, b_out)
    res = bass_utils.run_bass_kernel_spmd(
        nc,
        in_maps,
        core_ids=list(range(NCORES)),
        trace=trace,
        trace_cores=trace_cores,
    )
    outs = []
    for c in range(NCORES):
        oT = res.results[c]["outT"]                      # [BPC, OUT, N]
        outs.append(np.ascontiguousarray(oT.transpose(0, 2, 1)))
    full = np.concatenate(outs, axis=0).astype(np.float32)  # [B, N, OUT]
    return full, res


def kernel(inputs, supports, W_out, b_out):
    out, _ = run(inputs, supports, W_out, b_out, trace=False)
    return out


if __name__ == "__main__":
    # Smoke test in CoreSim (single core, core 0's shard).
    from concourse.bass_interp import CoreSim

    rng = np.random.default_rng(0)
    inputs = rng.standard_normal((B, N, D), dtype=np.float32)
    supports = (rng.random((S, B, N, N), dtype=np.float32) / N).astype(np.float32)
    W_out = (rng.standard_normal((D * NMET, OUT), dtype=np.float32) * 0.02).astype(
        np.float32
    )
    b_out = (rng.standard_normal((OUT,), dtype=np.float32) * 0.02).astype(np.float32)

    nc = _get_nc()
    in_maps = shard_inputs(inputs, supports, W_out, b_out)
    sim = CoreSim(nc, trace=False)
    for k, v in in_maps[0].items():
        sim.tensor(k)[:] = v
    sim.simulate(check_with_hw=False)
    got = sim.tensor("outT")[:].transpose(0, 2, 1)

    # numpy reference for core 0's batches
    x0 = inputs[:BPC].astype(np.float64)
    A = supports[:, :BPC].astype(np.float64)
    exp = []
    for b in range(BPC):
        m0 = x0[b]
        m1 = A[0, b] @ m0
        m2 = 2 * A[0, b] @ m1 - m0
        m3 = A[1, b] @ m2
        m4 = 2 * A[1, b] @ m3 - m1
        xcat = np.stack([m0, m1, m2, m3, m4], axis=-1).reshape(N, D * NMET)
        exp.append(xcat @ W_out.astype(np.float64) + b_out)
    exp = np.stack(exp)
    abserr = np.abs(got - exp)
    print("abs err max:", abserr.max(), "scale:", np.abs(exp).std())
    print("scale-relative:", abserr.max() / np.abs(exp).std())


# revision 8
# speedup vs baseline: 1.1843x; 1.1843x over previous
"""Batched graph-conv (diffusion GCN step) kernel for 8 TRN2 NeuronCores.

Math per batch b (B=64, N=512, D=64, S=2, MAX_STEP=2, M=5 metrics):
    m0 = x0
    m1 = A0 @ x0
    m2 = 2*A0 @ m1 - m0
    m3 = A1 @ m2
    m4 = 2*A1 @ m3 - m1
    out = sum_k  m_k @ W_k + b     where W_k = W_out.reshape(D,5,OUT)[:,k,:]

Sharding: batch dim across the 8 cores (8 batches/core); weights replicated.

Device dataflow (per batch):
  - A fed pre-transposed+tiled from host: aT[s,b][p, kc, i] = A[i, kc*128+p]
    so each k-chunk kc is a ready-to-stream matmul rhs [128(j), 512(i)].
  - state kept in BOTH forms: natural x[j,d] (matmul stationary operand)
    and transposed xT[d,i] (product-result layout + projection stream).
  - product:  psum[d,i] = sum_kc  x_nat[:,kc*D:].T @ aT[:,kc,:]   (f32r)
  - PE-transpose (identity trick) regenerates the natural form of m1..m3.
  - per-batch projection runs as one tight 5-matmul PSUM accumulation
    at the end (bank alive ~1us instead of the whole batch).
  - 4 batches in flight; per pipeline stage all 4 batches' product
    matmuls are emitted back-to-back to keep the PE dense (HAM warm);
    a dummy-transpose warmup covers the initial DMA window.
"""

import sys

import numpy as np

sys.path.insert(0, "/opt/trn_rl_repo")

from contextlib import ExitStack

import concourse.bass as bass
import concourse.mybir as mybir
import concourse.tile as tile
from concourse import bacc
from concourse import bass_utils

P = 128          # SBUF partitions / matmul contraction tile
N = 512          # graph nodes
D = 64           # feature dim
OUT = 64         # output dim
S = 2            # supports
NMET = 5         # metrics
B = 64           # global batch
NCORES = 8
BPC = B // NCORES  # batches per core
KC = N // P        # k-chunks per product
GRP = 2            # batches in flight per pipeline group

F32 = mybir.dt.float32
# f32r: fp32 storage, single-pass PE matmul (4x faster than fp32's 2-pass
# half-rate path when the moving dim is >= 256). Walrus requires every
# tensor consumed by an FP32r matmul to be *produced* as FP32r, so all
# matmul-feeding tensors (DRAM + SBUF) are declared float32r end-to-end.
RDT = mybir.dt.float32r


def build_nc():
    nc = bacc.Bacc("TRN2", target_bir_lowering=False, debug=False)

    xT_d = nc.dram_tensor("xT", [BPC, D, N], RDT, kind="ExternalInput").ap()
    xn_d = nc.dram_tensor("xnat", [BPC, P, KC * D], RDT, kind="ExternalInput").ap()
    a_d = nc.dram_tensor("aT", [S, BPC, P, KC, N], RDT, kind="ExternalInput").ap()
    w_d = nc.dram_tensor("w", [D, NMET * OUT], RDT, kind="ExternalInput").ap()
    b_d = nc.dram_tensor("bias", [OUT, 1], F32, kind="ExternalInput").ap()
    i_d = nc.dram_tensor("ident", [P, P], RDT, kind="ExternalInput").ap()
    o_d = nc.dram_tensor("outT", [BPC, OUT, N], F32, kind="ExternalOutput").ap()

    with ExitStack() as ctx:
        tc = ctx.enter_context(tile.TileContext(nc))
        _emit(nc, tc, ctx, xT_d, xn_d, a_d, w_d, b_d, i_d, o_d)

    nc.compile()
    return nc


def _emit(nc, tc, ctx, xT_d, xn_d, a_d, w_d, b_d, i_d, o_d):
    a_pool = ctx.enter_context(tc.tile_pool(name="a", bufs=12))
    stT = ctx.enter_context(tc.tile_pool(name="stT", bufs=5))
    nat = ctx.enter_context(tc.tile_pool(name="nat", bufs=5))
    outsb = ctx.enter_context(tc.tile_pool(name="outsb", bufs=3))
    const = ctx.enter_context(tc.tile_pool(name="const", bufs=1))
    ps_mm = ctx.enter_context(tc.tile_pool(name="psmm", bufs=4, space="PSUM"))
    ps_tr = ctx.enter_context(tc.tile_pool(name="pstr", bufs=2, space="PSUM"))
    ps_out = ctx.enter_context(tc.tile_pool(name="psout", bufs=2, space="PSUM"))

    w_sb = const.tile([D, NMET * OUT], RDT)
    nc.sync.dma_start(w_sb[:], w_d)
    bias_sb = const.tile([OUT, 1], F32)
    nc.sync.dma_start(bias_sb[:], b_d)
    ident = const.tile([P, P], RDT)
    nc.sync.dma_start(ident[:], i_d)

    def product(a_tile, src_nat, tag):
        """psum[d, i] = sum_j A[i, j] * x[j, d]  (result transposed)."""
        ps = ps_mm.tile([D, N], F32, tag="ps", name=f"ps_{tag}")
        for kc in range(KC):
            nc.tensor.matmul(
                ps[:],
                src_nat[:, kc * D:(kc + 1) * D],
                a_tile[:, kc, :],
                start=(kc == 0),
                stop=(kc == KC - 1),
            )
        return ps

    def transposes(mT, tag):
        """[D, N] transposed state -> PSUM [P, KC*D] natural (packed)."""
        pt = ps_tr.tile([P, KC * D], RDT, tag="pt", name=f"pt_{tag}")
        for kc in range(KC):
            nc.tensor.matmul(
                pt[:, kc * D:(kc + 1) * D],
                mT[:, kc * P:(kc + 1) * P],
                ident[:D, :D],
                is_transpose=True,
                skip_group_check=True,
            )
        return pt

    def evict_nat(pt, name):
        m_nat = nat.tile([P, KC * D], RDT, tag="nat", name=name, bufs=8)
        nc.scalar.copy(m_nat[:], pt[:])
        return m_nat

    # ---- PE warmup: dummy transposes on the identity while the first A
    # tiles stream in, so the HAM un-throttles before real work starts.
    # (borrows an out-psum slot, released before the first projection)
    warm_ps = ps_out.tile([P, KC * D], RDT, tag="outp", name="warm")
    for _ in range(48):
        nc.tensor.matmul(
            warm_ps[:, 0:D], ident[:D, 0:P], ident[:D, :D],
            is_transpose=True, skip_group_check=True,
        )

    # ---- input DMAs, emitted in exact use order (per pair of batches:
    # x tiles, then a0/a1 of each batch interleaved so every batch's full
    # 4-product chain unblocks as early as possible). The tile pools' buf
    # limits turn this into a continuous self-paced prefetch stream.
    loads = {}
    for g in range(0, BPC, GRP):
        for b in range(g, g + GRP):
            t = stT.tile([D, N], RDT, tag="x0T", name=f"x0T_{b}", bufs=6)
            nc.sync.dma_start(t[:], xT_d[b])
            loads[("x", b)] = t
            t = nat.tile([P, KC * D], RDT, tag="x0n", name=f"x0n_{b}", bufs=6)
            nc.sync.dma_start(t[:], xn_d[b])
            loads[("xn", b)] = t
        for b in range(g, g + GRP):
            for s in range(S):
                t = a_pool.tile([P, KC, N], RDT, tag="A", name=f"a{s}_{b}")
                nc.sync.dma_start(t[:], a_d[s, b])
                loads[("a", s, b)] = t

    def batch_stages(b):
        st = {}

        def load():
            st["x0T"] = loads[("x", b)]
            st["x0n"] = loads[("xn", b)]
            st["a0"] = loads[("a", 0, b)]
            st["a1"] = loads[("a", 1, b)]

        def p1():
            ps = product(st["a0"], st["x0n"], f"1_{b}")
            st["m1T"] = stT.tile([D, N], RDT, tag="m1T", name=f"m1T_{b}", bufs=4)
            nc.vector.tensor_copy(st["m1T"][:], ps[:])

        def t1():
            st["pt1"] = transposes(st["m1T"], f"1_{b}")
            st["m1n"] = evict_nat(st["pt1"], f"m1n_{b}")

        def p2():
            ps = product(st["a0"], st["m1n"], f"2_{b}")
            st["m2T"] = stT.tile([D, N], RDT, tag="m2T", name=f"m2T_{b}", bufs=4)
            nc.vector.scalar_tensor_tensor(
                st["m2T"][:], ps[:], 2.0, st["x0T"][:],
                op0=mybir.AluOpType.mult, op1=mybir.AluOpType.subtract,
            )

        def t2():
            st["pt2"] = transposes(st["m2T"], f"2_{b}")
            st["m2n"] = evict_nat(st["pt2"], f"m2n_{b}")

        def p3():
            ps = product(st["a1"], st["m2n"], f"3_{b}")
            st["m3T"] = stT.tile([D, N], RDT, tag="m3T", name=f"m3T_{b}", bufs=4)
            nc.vector.tensor_copy(st["m3T"][:], ps[:])

        def t3():
            st["pt3"] = transposes(st["m3T"], f"3_{b}")
            st["m3n"] = evict_nat(st["pt3"], f"m3n_{b}")

        def p4():
            ps = product(st["a1"], st["m3n"], f"4_{b}")
            st["m4T"] = stT.tile([D, N], RDT, tag="m4T", name=f"m4T_{b}", bufs=4)
            nc.vector.scalar_tensor_tensor(
                st["m4T"][:], ps[:], 2.0, st["m1T"][:],
                op0=mybir.AluOpType.mult, op1=mybir.AluOpType.subtract,
            )
            prj()

        def prj():
            outp = ps_out.tile([OUT, N], F32, tag="outp", name=f"outp_{b}")
            mts = [st["x0T"], st["m1T"], st["m2T"], st["m3T"], st["m4T"]]
            for k in range(NMET):
                nc.tensor.matmul(
                    outp[:],
                    w_sb[:, k * OUT:(k + 1) * OUT],
                    mts[k][:],
                    start=(k == 0),
                    stop=(k == NMET - 1),
                )
            o_sb = outsb.tile([OUT, N], F32, tag="o", name=f"o_{b}")
            nc.scalar.add(o_sb[:], outp[:], bias_sb[:])
            nc.gpsimd.dma_start(o_d[b], o_sb[:])

        return [load, p1, t1, p2, t2, p3, t3, p4]

    NSTAGE = 8
    for g in range(0, BPC, GRP):
        group = [batch_stages(b) for b in range(g, g + GRP)]
        for si in range(NSTAGE):
            for stages in group:
                stages[si]()


_NC_CACHE = None


def _get_nc():
    global _NC_CACHE
    if _NC_CACHE is None:
        _NC_CACHE = build_nc()
    return _NC_CACHE


def shard_inputs(inputs, supports, W_out, b_out):
    """Host-side shard + layout prep. Returns list of per-core input dicts."""
    inputs = np.ascontiguousarray(inputs, dtype=np.float32)
    supports = np.ascontiguousarray(supports, dtype=np.float32)
    w = np.ascontiguousarray(W_out.reshape(D, NMET * OUT), dtype=np.float32)
    bias = np.ascontiguousarray(b_out.reshape(OUT, 1), dtype=np.float32)
    ident = np.eye(P, dtype=np.float32)

    in_maps = []
    for c in range(NCORES):
        sl = slice(c * BPC, (c + 1) * BPC)
        x = inputs[sl]                                   # [BPC, N, D]
        xT = np.ascontiguousarray(x.transpose(0, 2, 1))  # [BPC, D, N]
        # natural packed: [p, kc*D + d] = x[kc*128 + p, d]
        xn = np.ascontiguousarray(
            x.reshape(BPC, KC, P, D).transpose(0, 2, 1, 3).reshape(BPC, P, KC * D)
        )
        a = supports[:, sl]                              # [S, BPC, N(i), N(j)]
        # aT[s,b][p, kc, i] = A[i, kc*128+p]
        aT = np.ascontiguousarray(
            a.transpose(0, 1, 3, 2)                      # [S, BPC, j, i]
            .reshape(S, BPC, KC, P, N)
            .transpose(0, 1, 3, 2, 4)                    # [S, BPC, p, kc, i]
        )
        in_maps.append({"xT": xT, "xnat": xn, "aT": aT, "w": w, "bias": bias,
                        "ident": ident})
    return in_maps


def run(inputs, supports, W_out, b_out, trace=False, trace_cores=None):
    nc = _get_nc()
    in_maps = shard_inputs(inputs, supports, W_out, b_out)
    res = bass_utils.run_bass_kernel_spmd(
        nc,
        in_maps,
        core_ids=list(range(NCORES)),
        trace=trace,
        trace_cores=trace_cores,
    )
    outs = []
    for c in range(NCORES):
        oT = res.results[c]["outT"]                      # [BPC, OUT, N]
        outs.append(np.ascontiguousarray(oT.transpose(0, 2, 1)))
    full = np.concatenate(outs, axis=0).astype(np.float32)  # [B, N, OUT]
    return full, res


def kernel(inputs, supports, W_out, b_out):
    out, _ = run(inputs, supports, W_out, b_out, trace=False)
    return out


if __name__ == "__main__":
    # Smoke test in CoreSim (single core, core 0's shard).
    from concourse.bass_interp import CoreSim

    rng = np.random.default_rng(0)
    inputs = rng.standard_normal((B, N, D), dtype=np.float32)
    supports = (rng.random((S, B, N, N), dtype=np.float32) / N).astype(np.float32)
    W_out = (rng.standard_normal((D * NMET, OUT), dtype=np.float32) * 0.02).astype(
        np.float32
    )
    b_out = (rng.standard_normal((OUT,), dtype=np.float32) * 0.02).astype(np.float32)

    nc = _get_nc()
    in_maps = shard_inputs(inputs, supports, W_out, b_out)
    sim = CoreSim(nc, trace=False)
    for k, v in in_maps[0].items():
        sim.tensor(k)[:] = v
    sim.simulate(check_with_hw=False)
    got = sim.tensor("outT")[:].transpose(0, 2, 1)

    # numpy reference for core 0's batches
    x0 = inputs[:BPC].astype(np.float64)
    A = supports[:, :BPC].astype(np.float64)
    exp = []
    for b in range(BPC):
        m0 = x0[b]
        m1 = A[0, b] @ m0
        m2 = 2 * A[0, b] @ m1 - m0
        m3 = A[1, b] @ m2
        m4 = 2 * A[1, b] @ m3 - m1
        xcat = np.stack([m0, m1, m2, m3, m4], axis=-1).reshape(N, D * NMET)
        exp.append(xcat @ W_out.astype(np.float64) + b_out)
    exp = np.stack(exp)
    abserr = np.abs(got - exp)
    print("abs err max:", abserr.max(), "scale:", np.abs(exp).std())
    print("scale-relative:", abserr.max() / np.abs(exp).std())
